# revision 75
# baseline (speedup 1.0000x reference)
"""GPT-2 style causal attention block (B=4, S=2048, E=1024, H=16, D=64) on
8 TRN2 NeuronCores.

Sharding: batch(4) x head-half(2) -> 8 cores, zero on-device communication.
Core c handles batch b=c//2 and heads h0=(c%2)*8 .. h0+7. Each core computes
its qkv column block, attention for its 8 heads, and a partial c_proj
(its 512 rows of w_proj). The two partial outputs per batch are summed on the
host during unshard (b_proj is given only to the even core of each pair).

v2 design (cost-model-driven; matmul cost = out free-size x cycles/row):
  qkv: fp8e4 DoubleRow (0.5 cyc/row, 2 contraction planes/inst). Host ships
  X^T and the x16-scaled weights as e4m3 hi/lo splits; V uses the 3-pass
  error-compensated product xh*wh + xh*wl + xl*wh (residual ~0.07%), Q/K use
  the 2-pass x*wh (the ~2.6% error only perturbs the exp argument; measured
  1.61e-2 total vs the 2e-2 gate on the fixed-seed reference). The x16 scale
  cancels via the exp scale (0.125/256) and a 16.0 ones-column in V.
  scores^T[k, q] per head in 1024-wide q-chunks, bf16 (contraction is 64:
  a single pass, so fp8 DoubleRow would not help); exp on ACT; causal
  diagonal masked by gpsimd affine_select AFTER the off-diagonal attn@V
  pieces so the exp->scores ladder is not gated by Pool.
  attn@V flipped: stationary = pt[:, qt-block], moving = V[kt] -> psA[q-part,
  qt, 64 d | 1 r] per half-chunk (65 cols/matmul instead of the q-width; the
  ones-column accumulates the softmax denominator). Normalize = per-partition
  reciprocal + broadcast-multiply -> at2 = A in [t, c] layout, then
  dma_start_transpose tiles into at[c, ct, t] for c_proj (bf16, 4-step
  contraction; tail rows 1024:2048 as two ct-pair partials in out2, summed on
  host). Scheduling: exp stream is the critical path (ACT ~158us busy);
  everything else (qkv units, V tiles, c_proj, out2 partials) is spread as
  per-(head, kt) PE filler so chunk-0 stays balanced and the last partial
  starts from head 7's early half-drain.
"""

import re

import ml_dtypes
import numpy as np

import concourse.mybir as mybir
import concourse.tile as tile
from concourse import bacc
from concourse.bass_utils import run_bass_kernel_spmd
from concourse.vector_clock import ScopedClock

F32 = mybir.dt.float32
BF16 = mybir.dt.bfloat16
FP8 = mybir.dt.float8e4
BF16_NP = ml_dtypes.bfloat16
FP8_NP = ml_dtypes.float8_e4m3
AF = mybir.ActivationFunctionType
DR = mybir.MatmulPerfMode.DoubleRow
WSCALE = 16.0  # qkv weights shipped x16 so the fp8 lo-residuals stay normal

S = 2048          # sequence length (per batch)
E = 1024          # embedding dim
HL = 8            # heads per core
D = 64            # head dim
TT = S // 128     # 16 token tiles
ET = E // 128     # 8 embedding tiles
NCH = S // 1024   # 2 q-chunks of 1024
PRIO_OFFSET = 4000  # attention body outranks ALL filler work


def _install_drain_fix():
    """walrus in this container rejects the Tile kernel-tail Drain when it
    carries all semaphore waits on one instruction ("Too many sync wait
    commands"). Emit one wait_ge per semaphore, then a bare drain."""
    if getattr(tile.TileContext, "_drain_fix_installed", False):
        return

    def _split_drain_and_barrier(self, tick_clock, wait_clock):
        nc = self.nc
        probe = mybir.InstDrain(
            name="probe-drain", engine=mybir.EngineType.SP, ins=[], outs=[]
        )
        wait_clock.add_sem_waits(probe, ScopedClock({None: tick_clock.global_clock}))
        waits = re.findall(r"wait:S\[([A-Za-z0-9_]+)\]>=(\d+)", probe.concise())
        handles = {h.name: h for h in self.sems.allocated().values()}
        for name, val in waits:
            nc.sync.wait_ge(handles[name], int(val))
        nc.sync.drain()
        nc.all_engine_barrier()
        popped = nc._tile_sem_poison_stack.pop()
        assert popped is self._sem_poison
        nc.clear_and_free_semaphores(list(self.sems.allocated().values()))
        nc.all_engine_barrier()

    tile.TileContext._drain_and_barrier = _split_drain_and_barrier
    tile.TileContext._drain_fix_installed = True


def _emit(nc, tc, ctx):
    # qkv operands arrive as fp8 hi/lo splits (x: unscaled, w: x16); the
    # 3-pass DoubleRow qkv computes xh*wh + xh*wl + xl*wh (error-compensated)
    xth_d = nc.declare_dram_parameter("xth", [E, S], FP8, isOutput=False)
    xtl_d = nc.declare_dram_parameter("xtl", [E, S], FP8, isOutput=False)
    wqkh_d = nc.declare_dram_parameter("wqkh", [E, 1024], FP8, isOutput=False)
    wvah_d = nc.declare_dram_parameter("wvah", [E, 512], FP8, isOutput=False)
    wval_d = nc.declare_dram_parameter("wval", [E, 512], FP8, isOutput=False)
    wp_d = nc.declare_dram_parameter("wp", [512, E], BF16, isOutput=False)
    bqk_d = nc.declare_dram_parameter("bqk", [8, 128, 1], F32, isOutput=False)
    bva_d = nc.declare_dram_parameter("bva", [1, 512], F32, isOutput=False)
    bp_d = nc.declare_dram_parameter("bp", [1, E], F32, isOutput=False)
    out_d = nc.declare_dram_parameter("out", [S, E], BF16, isOutput=True)
    # tail-region (rows 1024:2048) c_proj partials, one per ct PAIR;
    # summed on the host together with the core-pair reduction
    out2_d = nc.declare_dram_parameter("out2", [2, 1024, E], BF16, isOutput=True)

    consts = ctx.enter_context(tc.tile_pool(name="consts", bufs=1))
    statics = ctx.enter_context(tc.tile_pool(name="statics", bufs=1))
    ptp = ctx.enter_context(tc.tile_pool(name="ptp", bufs=10))
    rp = ctx.enter_context(tc.tile_pool(name="rp", bufs=4))
    # 6 bufs: a y tile lives ~2.7us (copy + DMA init + transfer); 3 bufs
    # paced the tail's out2 units at the ring, not the engines
    yp = ctx.enter_context(tc.tile_pool(name="yp", bufs=8))
    # PSUM budget (8 banks): sc 2x[128,1024]=4, a 2x[128,4,65]=2 (one per
    # half-chunk, ones-col carries the softmax denominator), qk 2x[128,512]=2
    psS = ctx.enter_context(tc.tile_pool(name="psS", bufs=2, space="PSUM"))
    psA = ctx.enter_context(tc.tile_pool(name="psA", bufs=2, space="PSUM"))
    psQ = ctx.enter_context(tc.tile_pool(name="psQ", bufs=2, space="PSUM"))

    # ---- front section: DMA order matters (the DMA engines are a single
    # serialized 360GB/s resource). X^T arrives host-pre-transposed; wqk
    # interleaves so qkv unlocks early; wp (needed last) at the end ----
    xth_sb = statics.tile([128, ET, S], FP8)
    xtl_sb = statics.tile([128, ET, S], FP8)
    wqkh_sb = statics.tile([128, ET, 1024], FP8)
    wvah_sb = statics.tile([128, ET, 512], FP8)
    wval_sb = statics.tile([128, ET, 512], FP8)
    wp_sb = statics.tile([128, 4, E], BF16)

    # DMA plan: two queues (SP=sync, Pool=gpsimd) drain in parallel; the exp
    # stream unblocks on {bqk, wqk m0/m4 slice, xth, xtl}, so those lead.
    # wqk columns are host-permuted to m-order (0,4,1,5,2,6,3,7) so head 0's
    # Q/K m-tiles are the first 256 columns and ship in one small DMA.
    # the ramp is DMA-serialization-bound: spread the X/wqk streams over FOUR
    # hardware queues (sync/scalar/vector/gpsimd). scalar/vector queue DMAs
    # finish before those engines have any work of their own.
    bqk_sb = consts.tile([128, 8], F32)
    nc.sync.dma_start(out=bqk_sb, in_=bqk_d.rearrange("m p one -> p (m one)"))
    xth_r = xth_d.rearrange("(e p) s -> p e s", p=128)
    xtl_r = xtl_d.rearrange("(e p) s -> p e s", p=128)
    for ep in range(2):
        es = slice(2 * ep, 2 * ep + 2)
        nc.sync.dma_start(out=xth_sb[:, es, :], in_=xth_r[:, es, :])
    nc.sync.dma_start(out=xtl_sb[:, 0:2, :], in_=xtl_r[:, 0:2, :])
    for ep in range(2, 4):
        es = slice(2 * ep, 2 * ep + 2)
        nc.scalar.dma_start(out=xth_sb[:, es, :], in_=xth_r[:, es, :])
    nc.scalar.dma_start(out=xtl_sb[:, 2:4, :], in_=xtl_r[:, 2:4, :])
    bva_st = consts.tile([1, 512], F32)
    nc.scalar.dma_start(out=bva_st, in_=bva_d[:])
    nc.sync.dma_start(
        out=wqkh_sb[:, :, 256:1024],
        in_=wqkh_d.rearrange("(e p) m -> p e m", p=128)[:, :, 256:1024],
    )
    bp_st = consts.tile([1, E], F32)
    nc.sync.dma_start(out=bp_st, in_=bp_d[:])
    nc.gpsimd.dma_start(
        out=wqkh_sb[:, :, 0:256],
        in_=wqkh_d.rearrange("(e p) m -> p e m", p=128)[:, :, 0:256],
    )
    for ep in range(2, 4):
        es = slice(2 * ep, 2 * ep + 2)
        nc.gpsimd.dma_start(out=xtl_sb[:, es, :], in_=xtl_r[:, es, :])
    # the broadcast is a Pool ENGINE op: it must not sit ahead of the xtl
    # stream in Pool's in-order queue (it waits on bva_st), but must precede
    # the first V drain
    bva_bc = consts.tile([128, 512], F32)
    nc.gpsimd.partition_broadcast(out_ap=bva_bc[:], in_ap=bva_st[:])
    nc.gpsimd.dma_start(
        out=wvah_sb, in_=wvah_d.rearrange("(e p) m -> p e m", p=128)
    )
    nc.gpsimd.dma_start(
        out=wval_sb, in_=wval_d.rearrange("(e p) m -> p e m", p=128)
    )
    bp_bc = consts.tile([128, E], F32)
    nc.gpsimd.partition_broadcast(out_ap=bp_bc[:], in_ap=bp_st[:])
    nc.gpsimd.dma_start(out=wp_sb, in_=wp_d.rearrange("(c p) m -> p c m", p=128))

    # ---- qkv Q^T,K^T (W stationary) paired so head h's Q and K m-tiles
    # arrive together, interleaved with V tiles -> attention starts early ----
    qkt_sb = statics.tile([128, 8, S], BF16)
    # V in [t, 520]: [64 d | 1] per head; the ones-columns (written once by a
    # strided memset) make attn@V also accumulate the softmax denominator
    va_sb = statics.tile([128, TT, HL * (D + 1)], BF16)
    at2_sb = statics.tile([128, TT, 512], BF16)  # A in [t, (h d)] layout
    at_sb = statics.tile([128, 4, S], BF16)      # A^T: rows c=h*64+d, cols t
    # ones-columns hold WSCALE so the x16 of the V d-columns cancels in the
    # normalize step (at2 = (16*sum P V) * 1/(16*sum P))
    nc.gpsimd.memset(
        va_sb[:, :, :].rearrange("p i (h c) -> p i h c", c=D + 1)[:, :, :, D : D + 1],
        WSCALE,
    )

    # (x, w) operand pairs for the 3-pass error-compensated fp8 product
    QKV_PASSES = [(xth_sb, "h"), (xth_sb, "l"), (xtl_sb, "h")]

    # physical column of logical m-tile in the host-permuted wqk layout
    MCOL = {0: 0, 4: 1, 1: 2, 5: 3, 2: 4, 6: 5, 3: 6, 7: 7}

    def emit_qk_tch(m, tch, pool=None, tag="qk"):
        pqk = (pool or psQ).tile([128, 512], F32, tag=tag, name="pqk")
        mc = MCOL[m]
        # Q/K tiles drop the x*w_lo pass: Q,K = X*Wh, whose ~2.6% errors
        # enter the softmax only through the exp argument (~1.5% on P,
        # measured 1.57e-2 total vs the 2e-2 gate on the fixed-seed
        # reference); V keeps full 3-pass compensation
        passes = [QKV_PASSES[0], QKV_PASSES[2]]
        npass = len(passes)
        k = 0
        for xsb, w in passes:
            assert w == "h"
            wsb = wqkh_sb
            for ep in range(ET // 2):
                nc.tensor.matmul(
                    pqk,
                    wsb[:, 2 * ep : 2 * ep + 2, mc * 128 : (mc + 1) * 128],
                    xsb[:, 2 * ep : 2 * ep + 2, tch * 512 : (tch + 1) * 512],
                    start=(k == 0),
                    stop=(k == npass * ET // 2 - 1),
                    perf_mode=DR,
                )
                k += 1
        # the PSUM->SBUF copy gates the consumer head's first scores: give it
        # FULL attention priority -- at head boundaries DVE must run it ahead
        # of the drains (which gate nothing on the exp ladder)
        with tc.high_priority(offset=PRIO_OFFSET):
            nc.vector.tensor_scalar_add(
                qkt_sb[:, m, tch * 512 : (tch + 1) * 512], pqk, bqk_sb[:, m : m + 1]
            )

    def emit_qk(m, ramp=False):
        # during the DMA-paced ramp the attention PSUM banks are still free:
        # spread the first pair's groups across them so more et-accumulations
        # are in flight per arriving weight tile
        pools = [psQ, psQ, psS, psA] if ramp else [psQ] * 4
        tags = ["qk", "qk", "sc", "a"] if ramp else ["qk"] * 4
        for tch in range(4):
            emit_qk_tch(m, tch, pool=pools[tch], tag=tags[tch])

    def emit_v(i, ramp=False):
        # never borrow psS: the first scores would queue behind the borrow
        pv1 = (psA if ramp else psQ).tile([128, 512], F32, tag="a" if ramp else "qk")
        k = 0
        for xsb, w in QKV_PASSES:
            wsb = wvah_sb if w == "h" else wval_sb
            for ep in range(ET // 2):
                nc.tensor.matmul(
                    pv1,
                    xsb[:, 2 * ep : 2 * ep + 2, i * 128 : (i + 1) * 128],
                    wsb[:, 2 * ep : 2 * ep + 2, :],
                    start=(k == 0),
                    stop=(k == 3 * ET // 2 - 1),
                    perf_mode=DR,
                )
                k += 1
        nc.vector.tensor_add(
            va_sb[:, i, :].rearrange("p (h c) -> p h c", c=D + 1)[:, :, 0:D],
            pv1[:, :].rearrange("p (h c) -> p h c", c=D),
            bva_bc[:, :].rearrange("p (h c) -> p h c", c=D),
        )

    # Minimal ramp: only what head 0 strictly needs up front (its Q/K m-tiles
    # and V tiles 0-3); everything else becomes in-loop PE filler so the exp
    # stream starts ~25us earlier
    # chunk-0 touches only tch 0-1 of Q and K (q in [0,1024), kt <= 7): the
    # ramp needs just head 0's four tch 0-1 units. tch 2-3 of every m-tile is
    # chunk-1-only work and is deferred there. Never borrow psS here -- the
    # first scores would wait for the borrowed bank's DVE drain; psA is safe
    # (first attn@V lands much later).
    ramp_pools = [psQ, psQ, psA, psA]
    ramp_tags = ["qk", "qk", "a", "a"]
    for u, (m, t) in enumerate([(0, 0), (0, 1), (4, 0), (4, 1)]):
        emit_qk_tch(m, t, pool=ramp_pools[u], tag=ramp_tags[u])
    emit_v(0)
    emit_v(1, ramp=True)
    emit_v(2)
    emit_v(3)

    def segs(off):
        if off < 512:
            return [(off, 512), (512, 1024)]
        return [(off, 1024)]

    def emit_tailB(i, full_borrow):
        # tail-region c_proj partial over ct pair 1 (ct 2-3) for token tile
        # 8+i. full_borrow=False while attention still runs (psQ + DVE only);
        # True after the last exp (all PSUM banks + the ACT engine free).
        y2 = yp.tile([128, E], BF16, tag="y", name="y2")
        for ech in range(2):
            u = (i - 8) * 2 + ech
            if full_borrow:
                pool, ptag = [(psQ, "qk"), (psQ, "qk"), (psS, "sc"), (psA, "a")][
                    u % 4
                ]
            else:
                pool, ptag = psQ, "qk"
            py = pool.tile([128, 512], F32, tag=ptag, name="py")
            for sub in range(2):
                ct = 2 + sub
                nc.tensor.matmul(
                    py,
                    at_sb[:, ct, i * 128 : (i + 1) * 128],
                    wp_sb[:, ct, ech * 512 : (ech + 1) * 512],
                    start=(sub == 0),
                    stop=(sub == 1),
                )
            if full_borrow and u % 2 == 0:
                nc.scalar.copy(out=y2[:, ech * 512 : (ech + 1) * 512], in_=py)
            else:
                nc.vector.tensor_copy(y2[:, ech * 512 : (ech + 1) * 512], py)
        nc.sync.dma_start(
            out=out2_d[1, (i - 8) * 128 : (i - 7) * 128, :], in_=y2
        )

    def emit_cproj(i):
        # rows i*128:(i+1)*128 of the output: full 4-ct contraction
        ysb = yp.tile([128, E], BF16, tag="y")
        for ech in range(2):
            py = psQ.tile([128, 512], F32, tag="qk")
            for ct in range(4):
                nc.tensor.matmul(
                    py,
                    at_sb[:, ct, i * 128 : (i + 1) * 128],
                    wp_sb[:, ct, ech * 512 : (ech + 1) * 512],
                    start=(ct == 0),
                    stop=(ct == 3),
                )
            nc.vector.tensor_add(
                ysb[:, ech * 512 : (ech + 1) * 512],
                py,
                bp_bc[:, ech * 512 : (ech + 1) * 512],
            )
            nc.sync.dma_start(
                out=out_d[i * 128 : (i + 1) * 128, ech * 512 : (ech + 1) * 512],
                in_=ysb[:, ech * 512 : (ech + 1) * 512],
            )

    # ---- attention (q-chunks of 1024): scores^T + exp as v1; attn@V flipped
    # so psA accumulates [q, qt, d] per head; r via ones-matmuls in psR ----
    for j in range(NCH):
        q0 = j * 1024
        nkt = 8 * (j + 1)

        # PE filler emitted between kt blocks (program order must place each
        # producer before its first consumer), spread so no head carries much
        # more PE work than one head's exp stream. tch 0-1 qkt units feed
        # chunk-0 consumers; tch 2-3 and V tiles 8-15 are chunk-1-only.
        if j == 0:
            fill0 = {(0, kt): ("v", 4 + kt) for kt in range(4)}
            fill0.update({
                (0, 4): ("qkt", (1, 0)), (0, 5): ("qkt", (5, 0)),
                (1, 0): ("qkt", (1, 1)), (1, 1): ("qkt", (5, 1)),
                (1, 4): ("qkt", (2, 0)), (1, 5): ("qkt", (6, 0)),
                (2, 0): ("qkt", (2, 1)), (2, 1): ("qkt", (6, 1)),
                (2, 4): ("qkt", (3, 0)), (2, 5): ("qkt", (7, 0)),
                (3, 0): ("qkt", (3, 1)), (3, 1): ("qkt", (7, 1)),
                # chunk-1 head 0 consumes Q m0 / K m4 tch 2-3 immediately:
                # finish them during the tail heads of chunk 0
                (6, 0): ("qkt", (0, 2)), (6, 1): ("qkt", (0, 3)),
                (6, 2): ("qkt", (4, 2)), (6, 3): ("qkt", (4, 3)),
                # V tiles 12-15 are needed late in chunk-1 head 0
                (7, 0): ("v", 12), (7, 1): ("v", 13),
                (7, 2): ("v", 14), (7, 3): ("v", 15),
            })
        else:
            fill0 = {(0, i): ("v", 8 + i) for i in range(4)}
            fill0.update({
                (1, 0): ("qkt", (1, 2)), (1, 1): ("qkt", (1, 3)),
                (1, 2): ("qkt", (5, 2)), (1, 3): ("qkt", (5, 3)),
                (3, 0): ("qkt", (2, 2)), (3, 1): ("qkt", (2, 3)),
                (3, 2): ("qkt", (6, 2)), (3, 3): ("qkt", (6, 3)),
                (5, 0): ("qkt", (3, 2)), (5, 1): ("qkt", (3, 3)),
                (5, 2): ("qkt", (7, 2)), (5, 3): ("qkt", (7, 3)),
                # early ct-pair-1 tail units right after head 7's half-0
                # drain (kt 11) -- only tiles 12-15 remain after the loop
                (7, 11): ("tbe", 8), (7, 12): ("tbe", 9),
                (7, 13): ("tbe", 10), (7, 14): ("tbe", 11),
            })

        for h in range(HL):
            po = (h % 2) * 64
            qm, km = h // 2, 4 + h // 2
            ctx_hp = tc.high_priority(offset=PRIO_OFFSET)
            ctx_hp.__enter__()
            # one psA bank per half-chunk of 4 q-tiles: [q, qt, 64 d | 1 r]
            pa = [psA.tile([128, 4, D + 1], F32, tag="a", name="pa") for _ in range(2)]
            # per-bank piece lists -> start/stop flags (first piece in a bank
            # marks the whole bank pending-zero, last carries stop)
            npc = [0, 0]
            for kt in range(nkt):
                qt0 = max(0, kt - 8 * j)
                for qt in range(qt0, 8):
                    npc[qt // 4] += 1
            idx = [0, 0]

            def drain(half):
                # per-partition reciprocal of the ones-column, then
                # broadcast-multiply along d into the A[t, c] tile
                rinv = rp.tile([128, 4], F32, tag="ri", name="rinv")
                nc.vector.reciprocal(out=rinv, in_=pa[half][:, :, D : D + 1])
                nc.vector.tensor_mul(
                    at2_sb[
                        :, j * 8 + 4 * half : j * 8 + 4 * (half + 1),
                        h * 64 : (h + 1) * 64,
                    ],
                    pa[half][:, :, 0:D],
                    rinv[:, :, None].broadcast_to((128, 4, D)),
                )

            for kt in range(nkt):
                p = kt - 8 * j
                off = max(0, p * 128)
                ps2 = psS.tile([128, 1024], F32, tag="sc")
                for a, b in segs(off):
                    nc.tensor.matmul(
                        ps2[:, a:b],
                        qkt_sb[po : po + 64, km, kt * 128 : (kt + 1) * 128],
                        qkt_sb[po : po + 64, qm, q0 + a : q0 + b],
                        start=True,
                        stop=True,
                    )
                pt = ptp.tile([128, 1024], BF16, tag="pt")
                # scores carry WSCALE^2 (Q and K both x16): fold 1/256 into
                # the exp scale together with 1/sqrt(D)
                nc.scalar.activation(
                    out=pt[:, off:1024],
                    in_=ps2[:, off:1024],
                    func=AF.Exp,
                    scale=0.125 / (WSCALE * WSCALE),
                )

                def av(qt):
                    half = qt // 4
                    # attn@V only gates the end-of-head drain, never the
                    # exp->scores ladder: run it at mid priority so the
                    # scheduler slots it into PE slack behind future scores
                    with tc.high_priority(offset=-PRIO_OFFSET // 2):
                        nc.tensor.matmul(
                            pa[half][:, qt % 4, :],
                            pt[:, qt * 128 : (qt + 1) * 128],
                            va_sb[:, kt, h * 65 : (h + 1) * 65],
                            start=(idx[half] == 0),
                            stop=(idx[half] == npc[half] - 1),
                        )
                    idx[half] += 1
                    if idx[half] == npc[half]:
                        drain(half)

                # off-diagonal attn@V first: only the diagonal piece waits on
                # the Pool-engine causal mask, so the exp->scores ladder for
                # kt+2 is not gated by affine_select
                for qt in range(max(0, p), 8):
                    if qt != p:
                        av(qt)
                if p >= 0:
                    # causal triangle on the diagonal 128-block: keep where
                    # q >= k, zero elsewhere (Pool engine; DVE is busier)
                    nc.gpsimd.affine_select(
                        out=pt[:, off : off + 128],
                        in_=pt[:, off : off + 128],
                        compare_op=mybir.AluOpType.is_ge,
                        fill=0.0,
                        base=0,
                        pattern=[[1, 128]],
                        channel_multiplier=-1,
                    )
                    av(p)
                if (h, kt) in fill0:
                    ctx_hp.__exit__(None, None, None)
                    kind, arg = fill0[(h, kt)]
                    if kind == "v":
                        emit_v(arg)
                    elif kind == "qkt":
                        emit_qk_tch(*arg)
                    else:  # "tbe": early ct-pair-1 tail unit
                        if arg == 8:
                            # at2 rows for tiles 8-11 (all heads, ct3 slice)
                            # are complete: head 7's half-0 drain just fired
                            for i in range(8, 12):
                                nc.sync.dma_start_transpose(
                                    out=at_sb[:, 3, i * 128 : (i + 1) * 128],
                                    in_=at2_sb[:, i, 384:512],
                                )
                        emit_tailB(arg, full_borrow=False)
                    ctx_hp = tc.high_priority(offset=PRIO_OFFSET)
                    ctx_hp.__enter__()
            ctx_hp.__exit__(None, None, None)
            if j != 0:
                if h == 0:
                    # chunk-0 A is complete: transpose tiles 0-7 into A^T
                    # (DMA xbar) for the main c_proj
                    for i in range(8):
                        nc.sync.dma_start_transpose(
                            out=at_sb[:, 0:4, i * 128 : (i + 1) * 128],
                            in_=at2_sb[:, i, :],
                        )
                if h % 2 == 1:
                    # tail-region (tiles 8-15) transposes for the ct slice of
                    # the head pair that just finished (ct3 tiles 8-11 are
                    # emitted early inside head 7's kt loop)
                    ct = h // 2
                    for i in range(12 if ct == 3 else 8, 16):
                        nc.sync.dma_start_transpose(
                            out=at_sb[:, ct, i * 128 : (i + 1) * 128],
                            in_=at2_sb[:, i, ct * 128 : (ct + 1) * 128],
                        )
                # chunk-0 c_proj tiles and ct-pair-0 tail units, spread so no
                # single head's PSUM-pool ring backs up
                for i in {1: [0], 2: [1, 2], 3: [3], 4: [4, 5], 5: [6],
                          6: [7]}.get(h, []):
                    emit_cproj(i)
                for i in {3: [8, 9], 4: [10, 11], 5: [12, 13],
                          6: [14, 15]}.get(h, []):
                    # tail-region c_proj partial over ct pair 0 (host sums
                    # the 2 partials)
                    y2 = yp.tile([128, E], BF16, tag="y")
                    for ech in range(2):
                        py = psQ.tile([128, 512], F32, tag="qk", name="py")
                        for sub in range(2):
                            nc.tensor.matmul(
                                py,
                                at_sb[:, sub, i * 128 : (i + 1) * 128],
                                wp_sb[:, sub, ech * 512 : (ech + 1) * 512],
                                start=(sub == 0),
                                stop=(sub == 1),
                            )
                        nc.vector.tensor_add(
                            y2[:, ech * 512 : (ech + 1) * 512],
                            py,
                            bp_bc[:, ech * 512 : (ech + 1) * 512],
                        )
                    nc.sync.dma_start(
                        out=out2_d[0, (i - 8) * 128 : (i - 7) * 128, :],
                        in_=y2,
                    )
                if h == 7:
                    # remaining ct-pair-1 tail units (tiles 12-15): after the
                    # last exp they may borrow all PSUM banks + the ACT engine
                    for i in range(12, 16):
                        emit_tailB(i, full_borrow=True)


def build_nc():
    _install_drain_fix()
    from contextlib import ExitStack

    nc = bacc.Bacc()
    with ExitStack() as ctx:
        tc = ctx.enter_context(tile.TileContext(nc))
        _emit(nc, tc, ctx)
    nc.finalize()  # Bacc: alloc_regs + insert_library_loads happen here
    return nc


def make_in_maps(inputs, w_attn, b_attn, w_proj, b_proj):
    """Build the 8 per-core input dicts from the full tensors.
    X / weights / mask go down pre-converted to bf16 (the compute dtype)."""
    x = np.asarray(inputs, dtype=np.float32)
    w_attn = np.asarray(w_attn, dtype=np.float32)
    b_attn = np.asarray(b_attn, dtype=np.float32)
    w_proj = np.asarray(w_proj, dtype=np.float32)
    b_proj = np.asarray(b_proj, dtype=np.float32)

    def split8(a):
        hi = a.astype(FP8_NP)
        lo = (a - hi.astype(np.float32)).astype(FP8_NP)
        return np.ascontiguousarray(hi), np.ascontiguousarray(lo)

    in_maps = []
    for c in range(8):
        b, half = c // 2, c % 2
        h0 = half * 8
        cols = np.arange(h0 * 64, h0 * 64 + 512)
        # qkv weights x16 so the fp8 lo-residual stays in e4m3's normal range;
        # columns permuted to m-order (0,4,1,5,2,6,3,7): Q/K m-tile pairs
        # adjacent so head 0's weights ship in one small leading DMA
        wq = WSCALE * w_attn[:, cols]
        wk = WSCALE * w_attn[:, 1024 + cols]
        mtiles = []
        for mi in range(4):
            mtiles.append(wq[:, mi * 128 : (mi + 1) * 128])
            mtiles.append(wk[:, mi * 128 : (mi + 1) * 128])
        wqk = np.concatenate(mtiles, axis=1)
        bqk = WSCALE * np.concatenate(
            [b_attn[cols], b_attn[1024 + cols]]
        ).reshape(8, 128, 1)
        vbase = 2048 + h0 * 64
        wva = WSCALE * w_attn[:, vbase : vbase + 512]
        bva = WSCALE * b_attn[vbase : vbase + 512].reshape(1, 512)
        wp = np.ascontiguousarray(w_proj[h0 * 64 : h0 * 64 + 512, :].astype(BF16_NP))
        bp = (b_proj if half == 0 else np.zeros_like(b_proj)).reshape(1, E)
        xth, xtl = split8(x[b].T)
        wqkh = np.ascontiguousarray(wqk.astype(FP8_NP))
        wvah, wval = split8(wva)
        in_maps.append(
            {
                "xth": xth,
                "xtl": xtl,
                "wqkh": wqkh,
                "wvah": wvah,
                "wval": wval,
                "wp": wp,
                "bqk": np.ascontiguousarray(bqk.astype(np.float32)),
                "bva": np.ascontiguousarray(bva.astype(np.float32)),
                "bp": np.ascontiguousarray(bp.astype(np.float32)),
            }
        )
    return in_maps


_CACHE = {}


def kernel(**inputs):
    nc = _CACHE.get("nc")
    if nc is None:
        nc = _CACHE["nc"] = build_nc()
    in_maps = make_in_maps(
        inputs["inputs"],
        inputs["w_attn"],
        inputs["b_attn"],
        inputs["w_proj"],
        inputs["b_proj"],
    )
    res = run_bass_kernel_spmd(nc, in_maps, core_ids=list(range(8)))
    return gather(res.results)


def gather(results):
    out = np.zeros((4, S, E), dtype=np.float32)
    for b in range(4):
        for c in (2 * b, 2 * b + 1):
            r = results[c]
            # rows 0:1024 come from "out"; the device writes rows 1024:2048
            # only via the per-ct partials in "out2"
            out[b, 0:1024] += r["out"][0:1024].astype(np.float32)
            out[b, 1024:2048] += r["out2"].astype(np.float32).sum(axis=0)
    return out
